# revision 9
# baseline (speedup 1.0000x reference)
"""Trainium2 Bass kernel for single-head causal attention.

Problem: x[4096,2048]; q/k/v = x@W + b; scores = causal(q k^T / sqrt(d_head));
out = softmax(scores) @ v @ W_O + b_O.

Strategy (8 NeuronCores, SPMD, one AllGather):
  * V path folded away on host: P @ (x W_V) W_O = (P @ x) @ (W_V W_O), so the
    device never projects V and never gathers it -- only K moves between
    cores.
  * K projection sharded: core c computes k^T for keys [512c, 512(c+1)),
    AllGathers the 8 shards (16MB, ~266us modeled). Under the collective
    every core redundantly recomputes key blocks 0-2, its Q projection and
    the first 12 score tiles, keeping the PE busy for ~250us of the window.
  * Causal interleave: core c owns query chunks {8j+c : j<4} of 128 rows.
    Chunk j only attends keys [0, 1024(j+1)) -- the same static extent on
    every core, so the SPMD program skips 37.5% of the attention FLOPs.
  * Scores computed transposed (scoresT[key, q] via lhsT=k^T tile), so the
    exp'd weights are directly the lhsT of the (P @ x) contraction -- no PE
    transposes anywhere. Row sums via ones-vector matmuls; 1/rowsum applied
    on the final PSUM->SBUF copy. wT is packed to its 80 live 128-col blocks.
  * DMA pacing: prefetches are emitted interleaved with the ks shard writes
    so the (serialized) DMA engines grant the AllGather inputs first, and
    every later phase finds its inputs already resident.

  Numerics: bf16 matmul inputs, f32 PSUM. 1/sqrt(d_head) folded into W_Q,
  constant-max softmax exp(s-25). b_K is softmax-invariant (row-constant
  shift); b_V/b_O folded on host; b_Q asserted zero.
"""

import math
import os
import sys

for _p in ("/opt/trn_rl_repo",):
    if _p not in sys.path and os.path.isdir(_p):
        sys.path.insert(0, _p)

import numpy as np
import ml_dtypes

import concourse.bass as bass
import concourse.mybir as mybir
import concourse.tile as tile
from concourse import bass_utils
from contextlib import ExitStack

P = 128
S = 4096
D = 2048
R = 512          # query rows per core
DT = D // P      # 16 d tiles
KT = S // P      # 32 key tiles
NCH = R // P     # 4 query chunks per core
LKB = 3          # key blocks of 512 computed locally (keys [0, 1536))
NB = 512
BF16 = mybir.dt.bfloat16
F32 = mybir.dt.float32
AF = mybir.ActivationFunctionType
EXP_SHIFT = -25.0  # constant-max softmax shift; |scores| << 25 for this data

# packed wT block offsets: tile t holds chunks t//8..3, each 128 cols
OFFS = []
_o = 0
for _t in range(KT):
    OFFS.append(_o)
    _o += NCH - _t // 8
N_WBLK = _o  # 80

LAST_RESULT = None  # test.py reads exec_time_ns from here


def split_multi_waits(nc):
    """This neuronxcc walrus lowers at most ONE sync wait per instruction
    (setupSyncWait: 'Too many sync wait commands'). Tile emits multi-wait
    instructions; hoist all but the last wait onto preceding EventSemaphore
    instructions on the same engine (strictly more conservative ordering)."""
    n_split = 0

    def fix(blocks):
        nonlocal n_split
        for b in blocks:
            out = []
            changed = False
            for inst in b.instructions:
                si = inst.sync_info
                waits = list(si.on_wait) if si is not None and si.on_wait else []
                if len(waits) > 1:
                    for j, w in enumerate(waits[:-1]):
                        es = mybir.InstEventSemaphore(
                            name=f"{inst.name}-esw{j}", ins=[], outs=[])
                        es.engine = inst.engine
                        es.sync_info = mybir.SyncInfo(on_wait=[w], on_update=[])
                        out.append(es)
                        n_split += 1
                    inst.sync_info = mybir.SyncInfo(
                        on_wait=[waits[-1]],
                        on_update=list(si.on_update) if si.on_update else [])
                    changed = True
                out.append(inst)
            if changed:
                b.instructions = out

    for fn in nc.m.functions:
        fix(fn.blocks)
    return n_split


def build_bass(n_cores=8, trace_label=""):
    nc = bass.Bass("TRN2", target_bir_lowering=False, debug=False,
                   enable_asserts=False, num_devices=n_cores)

    xq_d = nc.dram_tensor("xq", [D, R], BF16, kind="ExternalInput").ap()
    xkT_d = nc.dram_tensor("xkT", [D, NB], BF16, kind="ExternalInput").ap()
    xkl_d = nc.dram_tensor("xkl", [D, LKB * NB], BF16, kind="ExternalInput").ap()
    wq_d = nc.dram_tensor("wq", [D, D], BF16, kind="ExternalInput").ap()
    wk_d = nc.dram_tensor("wk", [D, D], BF16, kind="ExternalInput").ap()
    wvo_d = nc.dram_tensor("wvo", [D, D], BF16, kind="ExternalInput").ap()
    # xn pre-shuffled on host: xn_d[128m+p, 128o+n] = x[128o+p, 128m+n]
    xn_d = nc.dram_tensor("xn", [D, S], BF16, kind="ExternalInput").ap()
    # mask pre-shuffled: mask_d[s, 128t+i] = causal mask for key-tile t,
    # key-in-tile s, chunk-(t//8) query column i (per-core data)
    mask_d = nc.dram_tensor("mask", [P, KT * P], BF16, kind="ExternalInput").ap()
    out_d = nc.dram_tensor("out", [R, D], F32, kind="ExternalOutput").ap()

    def colb(ap_2d, j0, w):
        # DRAM [A, B] column slice [:, j0:j0+w] -> SBUF layout [P, A//P, w]
        return ap_2d[:, j0:j0 + w].rearrange("(o p) n -> p o n", p=P)

    with ExitStack() as ctx:
        tc = ctx.enter_context(tile.TileContext(nc))
        ps_mm = ctx.enter_context(tc.tile_pool(name="ps_mm", bufs=4, space="PSUM"))
        ps_px = ctx.enter_context(tc.tile_pool(name="ps_px", bufs=2, space="PSUM"))
        ps_rs = ctx.enter_context(tc.tile_pool(name="ps_rs", bufs=2, space="PSUM"))
        persist = ctx.enter_context(tc.tile_pool(name="persist", bufs=1))
        dram = ctx.enter_context(tc.tile_pool(name="dram", bufs=1, space="DRAM"))
        stage = ctx.enter_context(tc.tile_pool(name="stage", bufs=2))

        qT = persist.tile([P, DT, R], BF16, tag="qT")
        expb = persist.tile([P, 1], F32, tag="expb")
        nc.vector.memset(expb, EXP_SHIFT)
        ones = persist.tile([P, 1], BF16, tag="ones")
        nc.vector.memset(ones, 1.0)
        rsum = persist.tile([P, NCH], F32, tag="rsum")
        rrec = persist.tile([P, NCH], F32, tag="rrec")

        ks = dram.tile([D, NB], BF16, tag="ks")
        ktg = dram.tile([n_cores * D, NB], BF16, tag="ktg")

        # long-lived mid pool: K-local outputs + mask + first wk reload block
        mid = ctx.enter_context(tc.tile_pool(name="mid", bufs=1))
        kT01 = mid.tile([P, DT, 2 * NB], BF16, tag="kT01")
        blk2 = mid.tile([P, DT, NB], BF16, tag="blk2")
        maskT = mid.tile([P, KT, P], BF16, tag="maskT")
        wkl0 = mid.tile([P, DT, NB], BF16, tag="wkl0")

        # xkl: prefetched under phase 1, consumed by phase 2
        xklp_cm = tc.tile_pool(name="xklp", bufs=1)
        xklp = xklp_cm.__enter__()
        xkl = xklp.tile([P, DT, LKB * NB], BF16, tag="xkl")

        # prefetch chunks paced between ks shard writes (one per write):
        # the xkl pieces and the first wk reload block.
        def pf_emit(i):
            if i < LKB:
                nc.sync.dma_start(xkl[:, :, i * NB:(i + 1) * NB],
                                  colb(xkl_d, i * NB, NB))
            elif i == LKB:
                nc.sync.dma_start(wkl0, colb(wk_d, 0, NB))

        # ---------------- phase 1: K shard -> DRAM, then AllGather ---------
        with tc.tile_pool(name="p1", bufs=2) as p1, \
             tc.tile_pool(name="p1x", bufs=1) as p1x:
            xkT = p1x.tile([P, DT, NB], BF16, tag="xkT")
            for k in range(DT):
                nc.sync.dma_start(xkT[:, k, :], xkT_d[k * P:(k + 1) * P, :])
            for mb in range(4):
                wkb = p1.tile([P, DT, NB], BF16, tag="wkb")
                if mb == 0:
                    for k in range(DT):
                        nc.sync.dma_start(wkb[:, k, :], wk_d[k * P:(k + 1) * P, 0:NB])
                else:
                    nc.sync.dma_start(wkb, colb(wk_d, mb * NB, NB))
                for mm in range(4):
                    m = 4 * mb + mm
                    ps = ps_mm.tile([P, NB], F32, tag="mm")
                    for k in range(DT):
                        nc.tensor.matmul(ps, wkb[:, k, mm * P:(mm + 1) * P],
                                         xkT[:, k, :],
                                         start=(k == 0), stop=(k == DT - 1))
                    st = stage.tile([P, NB], BF16, tag="stg")
                    nc.scalar.activation(st, ps, AF.Copy)
                    nc.sync.dma_start(ks[m * P:(m + 1) * P, :], st)
                    pf_emit(m)

        nc.gpsimd.collective_compute(
            "AllGather", mybir.AluOpType.bypass,
            replica_groups=[list(range(n_cores))],
            ins=[ks.opt()], outs=[ktg.opt()],
        )

        # ---------------- phase 2 (under AG): local K blocks 0..2 ----------
        with tc.tile_pool(name="wklp", bufs=2) as wklp:
            for mb in range(4):
                if mb == 0:
                    wkb = wkl0
                else:
                    wkb = wklp.tile([P, DT, NB], BF16, tag="wkl")
                    nc.sync.dma_start(wkb, colb(wk_d, mb * NB, NB))
                for mm in range(4):
                    m = 4 * mb + mm
                    for h in range(LKB):
                        ps = ps_mm.tile([P, NB], F32, tag="mm")
                        for k in range(DT):
                            nc.tensor.matmul(ps, wkb[:, k, mm * P:(mm + 1) * P],
                                             xkl[:, k, h * NB:(h + 1) * NB],
                                             start=(k == 0), stop=(k == DT - 1))
                        dst = kT01[:, m, h * NB:(h + 1) * NB] if h < 2 \
                            else blk2[:, m, :]
                        nc.scalar.activation(dst, ps, AF.Copy)
        xklp_cm.__exit__(None, None, None)

        # ---------------- phase 3 (under AG): Q projection -----------------
        with tc.tile_pool(name="p3", bufs=2) as p3, \
             tc.tile_pool(name="p3x", bufs=1) as p3x:
            xq = p3x.tile([P, DT, R], BF16, tag="xq")
            nc.sync.dma_start(xq, xq_d.rearrange("(o p) n -> p o n", p=P))
            for mb in range(4):
                wqb = p3.tile([P, DT, NB], BF16, tag="wqb")
                nc.sync.dma_start(wqb, colb(wq_d, mb * NB, NB))
                for mm in range(4):
                    m = 4 * mb + mm
                    ps = ps_mm.tile([P, NB], F32, tag="mm")
                    for k in range(DT):
                        nc.tensor.matmul(ps, wqb[:, k, mm * P:(mm + 1) * P],
                                         xq[:, k, :],
                                         start=(k == 0), stop=(k == DT - 1))
                    nc.scalar.activation(qT[:, m, :], ps, AF.Copy)

        # remaining prefetches (issue under AG)
        nc.sync.dma_start(maskT, mask_d.rearrange("p (o n) -> p o n", n=P))
        xnp = ctx.enter_context(tc.tile_pool(name="xnp", bufs=2))
        xn_pre = []
        for m in range(2):
            xb = xnp.tile([P, KT, P], BF16, tag="xn")
            nc.sync.dma_start(
                xb, xn_d[m * P:(m + 1) * P, :].rearrange("p (o n) -> p o n", n=P))
            xn_pre.append(xb)

        late = ctx.enter_context(tc.tile_pool(name="late", bufs=1))
        wT = late.tile([P, N_WBLK * P], BF16, tag="wT")
        pxT = late.tile([P, DT, R], BF16, tag="pxT")

        # staging for gathered kT blocks (b >= 3)
        p4k_cm = tc.tile_pool(name="p4k", bufs=3)
        p4k = p4k_cm.__enter__()

        # ---------------- phase 4: scoresT -> exp -> mask, + row sums ------
        # scoresT[key, q] per key-tile t; chunk j attends tiles t < 8(j+1),
        # so tile t covers query columns [128*(t//8), 512), stored packed at
        # wT column block OFFS[t].
        ktb = None
        for t in range(KT):
            if t >= 4 * LKB and t % 4 == 0:
                b = t // 4
                ktb = p4k.tile([P, DT, NB], BF16, tag="ktb")
                nc.sync.dma_start(
                    ktb, ktg[b * D:(b + 1) * D, :].rearrange("(o p) n -> p o n", p=P))
            q0 = (t // 8) * P
            w = R - q0
            c0 = OFFS[t] * P
            ps = ps_mm.tile([P, NB], F32, tag="mm")
            for k in range(DT):
                if t < 8:
                    lhs = kT01[:, k, t * P:(t + 1) * P]
                elif t < 4 * LKB:
                    lhs = blk2[:, k, (t % 4) * P:(t % 4 + 1) * P]
                else:
                    lhs = ktb[:, k, (t % 4) * P:(t % 4 + 1) * P]
                nc.tensor.matmul(ps[:, :w], lhs, qT[:, k, q0:R],
                                 start=(k == 0), stop=(k == DT - 1))
            nc.scalar.activation(wT[:, c0:c0 + w], ps[:, :w], AF.Exp, bias=expb)
            nc.vector.tensor_mul(wT[:, c0:c0 + P], wT[:, c0:c0 + P], maskT[:, t, :])
            if t % 8 == 7:
                j = t // 8
                nt = 8 * (j + 1)
                psr = ps_rs.tile([P, 1], F32, tag="rs")
                for tt in range(nt):
                    cb = (OFFS[tt] + j - tt // 8) * P
                    nc.tensor.matmul(psr, wT[:, cb:cb + P], ones,
                                     start=(tt == 0), stop=(tt == nt - 1))
                nc.scalar.activation(rsum[:, j:j + 1], psr, AF.Copy)
        nc.vector.reciprocal(rrec, rsum)
        p4k_cm.__exit__(None, None, None)

        # ---------------- phase 5: pxT = (weights @ x)^T -------------------
        for m in range(DT):
            if m < 2:
                xb = xn_pre[m]
            else:
                xb = xnp.tile([P, KT, P], BF16, tag="xn")
                nc.sync.dma_start(
                    xb, xn_d[m * P:(m + 1) * P, :].rearrange("p (o n) -> p o n", n=P))
            for j in range(NCH):
                nt = 8 * (j + 1)
                ps = ps_px.tile([P, P], F32, tag="px")
                for t in range(nt):
                    cb = (OFFS[t] + j - t // 8) * P
                    nc.tensor.matmul(ps, xb[:, t, :], wT[:, cb:cb + P],
                                     start=(t == 0), stop=(t == nt - 1))
                nc.scalar.activation(pxT[:, m, j * P:(j + 1) * P], ps, AF.Copy)

        # ---------------- phase 6: out = pxT^T @ W_VO, scaled by 1/rowsum --
        with tc.tile_pool(name="wvop", bufs=2) as wvop, \
             tc.tile_pool(name="p7s", bufs=2) as p7s:
            wvo_pre = []
            for nb in range(2):
                wb = wvop.tile([P, DT, NB], BF16, tag="wvo")
                nc.sync.dma_start(wb, colb(wvo_d, nb * NB, NB))
                wvo_pre.append(wb)
            for nb in range(4):
                if nb < 2:
                    wb = wvo_pre[nb]
                else:
                    wb = wvop.tile([P, DT, NB], BF16, tag="wvo")
                    nc.sync.dma_start(wb, colb(wvo_d, nb * NB, NB))
                for j in range(NCH):
                    ps = ps_mm.tile([P, NB], F32, tag="mm")
                    for m in range(DT):
                        nc.tensor.matmul(ps, pxT[:, m, j * P:(j + 1) * P],
                                         wb[:, m, :],
                                         start=(m == 0), stop=(m == DT - 1))
                    ost = p7s.tile([P, NB], F32, tag="ost")
                    nc.scalar.activation(ost, ps, AF.Copy, scale=rrec[:, j:j + 1])
                    nc.sync.dma_start(
                        out_d[j * P:(j + 1) * P, nb * NB:(nb + 1) * NB], ost)

    split_multi_waits(nc)
    return nc


def _rows_of(c):
    return np.concatenate(
        [np.arange(P * (8 * j + c), P * (8 * j + c) + P) for j in range(NCH)])


def _mask_for(c):
    tiles = np.zeros((KT, P, P), dtype=np.float32)
    tri = np.triu(np.ones((P, P), dtype=np.float32))  # [s, i]: 1 iff s <= i
    for t in range(KT):
        u = t % 8
        if u < c:
            tiles[t] = 1.0
        elif u == c:
            tiles[t] = tri
    # -> mask_d[s, 128t + i]
    return tiles.transpose(1, 0, 2).reshape(P, KT * P)


def kernel(x, W_Q, W_K, W_V, W_O, b_Q, b_K, b_V, b_O, d_head, trace=False):
    global LAST_RESULT
    x = np.asarray(x, np.float32)
    n_cores = 8
    dh = float(np.asarray(d_head))
    scale = 1.0 / math.sqrt(dh)
    assert x.shape == (S, D)
    bq = np.asarray(b_Q, np.float32)
    assert not np.any(bq), "b_Q != 0 not supported by this kernel"
    # b_K shifts every score in a row by the same q_i.b_K: softmax-invariant.

    bf = ml_dtypes.bfloat16
    xT_b = np.ascontiguousarray(x.T).astype(bf)                      # [D, S]
    x_b = x.astype(bf)                                               # [S, D]
    wq_b = (np.asarray(W_Q, np.float32) * scale).astype(bf)
    wk_b = np.asarray(W_K, np.float32).astype(bf)
    wvo_b = (np.asarray(W_V, np.float32) @ np.asarray(W_O, np.float32)).astype(bf)
    # xn_d[128m+p, 128o+n] = x[128o+p, 128m+n]
    xn_sh = np.ascontiguousarray(
        x_b.reshape(KT, P, DT, P).transpose(2, 1, 0, 3).reshape(D, S))

    in_maps = []
    for c in range(n_cores):
        in_maps.append({
            "xq": np.ascontiguousarray(xT_b[:, _rows_of(c)]),
            "xkT": np.ascontiguousarray(xT_b[:, c * NB:(c + 1) * NB]),
            "xkl": np.ascontiguousarray(xT_b[:, :LKB * NB]),
            "wq": wq_b, "wk": wk_b, "wvo": wvo_b,
            "xn": xn_sh,
            "mask": _mask_for(c).astype(bf),
        })

    nc = build_bass(n_cores)
    res = bass_utils.run_bass_kernel_spmd(nc, in_maps, core_ids=list(range(n_cores)),
                                          trace=trace)
    LAST_RESULT = res
    out = np.empty((S, D), dtype=np.float32)
    for c in range(n_cores):
        out[_rows_of(c)] = res.results[c]["out"]
    # b_V/b_O fold linearly into the output (softmax rows sum to 1).
    out += (np.asarray(b_V, np.float32) @ np.asarray(W_O, np.float32)
            + np.asarray(b_O, np.float32))[None, :]
    return out


# revision 11
# speedup vs baseline: 1.0640x; 1.0640x over previous
"""Trainium2 Bass kernel for single-head causal attention.

Problem: x[4096,2048]; q/k/v = x@W + b; scores = causal(q k^T / sqrt(d_head));
out = softmax(scores) @ v @ W_O + b_O.

Strategy (8 NeuronCores, SPMD, one AllGather):
  * V path folded away on host: P @ (x W_V) W_O = (P @ x) @ (W_V W_O), so the
    device never projects V and never gathers it -- only K moves between
    cores.
  * K projection sharded: core c computes k^T for keys [512c, 512(c+1)),
    AllGathers the 8 shards (16MB, ~266us modeled). Under the collective
    every core redundantly recomputes key blocks 0-2, its Q projection and
    the first 12 score tiles, keeping the PE busy for ~250us of the window.
  * Causal interleave: core c owns query chunks {8j+c : j<4} of 128 rows.
    Chunk j only attends keys [0, 1024(j+1)) -- the same static extent on
    every core, so the SPMD program skips 37.5% of the attention FLOPs.
  * Scores computed transposed (scoresT[key, q] via lhsT=k^T tile), so the
    exp'd weights are directly the lhsT of the (P @ x) contraction -- no PE
    transposes anywhere. Row sums via ones-vector matmuls; 1/rowsum applied
    on the final PSUM->SBUF copy. wT is packed to its 80 live 128-col blocks.
  * DMA pacing: prefetches are emitted interleaved with the ks shard writes
    so the (serialized) DMA engines grant the AllGather inputs first, and
    every later phase finds its inputs already resident.

  Numerics: bf16 matmul inputs, f32 PSUM. 1/sqrt(d_head) folded into W_Q,
  constant-max softmax exp(s-25). b_K is softmax-invariant (row-constant
  shift); b_V/b_O folded on host; b_Q asserted zero.
"""

import math
import os
import sys

for _p in ("/opt/trn_rl_repo",):
    if _p not in sys.path and os.path.isdir(_p):
        sys.path.insert(0, _p)

import numpy as np
import ml_dtypes

import concourse.bass as bass
import concourse.mybir as mybir
import concourse.tile as tile
from concourse import bass_utils
from contextlib import ExitStack

P = 128
S = 4096
D = 2048
R = 512          # query rows per core
DT = D // P      # 16 d tiles
KT = S // P      # 32 key tiles
NCH = R // P     # 4 query chunks per core
LKB = 3          # key blocks of 512 computed locally (keys [0, 1536))
NB = 512
BF16 = mybir.dt.bfloat16
F32 = mybir.dt.float32
AF = mybir.ActivationFunctionType
EXP_SHIFT = -25.0  # constant-max softmax shift; |scores| << 25 for this data

# packed wT block offsets: tile t holds chunks t//8..3, each 128 cols
OFFS = []
_o = 0
for _t in range(KT):
    OFFS.append(_o)
    _o += NCH - _t // 8
N_WBLK = _o  # 80

LAST_RESULT = None  # test.py reads exec_time_ns from here


def split_multi_waits(nc):
    """This neuronxcc walrus lowers at most ONE sync wait per instruction
    (setupSyncWait: 'Too many sync wait commands'). Tile emits multi-wait
    instructions; hoist all but the last wait onto preceding EventSemaphore
    instructions on the same engine (strictly more conservative ordering)."""
    n_split = 0

    def fix(blocks):
        nonlocal n_split
        for b in blocks:
            out = []
            changed = False
            for inst in b.instructions:
                si = inst.sync_info
                waits = list(si.on_wait) if si is not None and si.on_wait else []
                if len(waits) > 1:
                    for j, w in enumerate(waits[:-1]):
                        es = mybir.InstEventSemaphore(
                            name=f"{inst.name}-esw{j}", ins=[], outs=[])
                        es.engine = inst.engine
                        es.sync_info = mybir.SyncInfo(on_wait=[w], on_update=[])
                        out.append(es)
                        n_split += 1
                    inst.sync_info = mybir.SyncInfo(
                        on_wait=[waits[-1]],
                        on_update=list(si.on_update) if si.on_update else [])
                    changed = True
                out.append(inst)
            if changed:
                b.instructions = out

    for fn in nc.m.functions:
        fix(fn.blocks)
    return n_split


def build_bass(n_cores=8, trace_label=""):
    nc = bass.Bass("TRN2", target_bir_lowering=False, debug=False,
                   enable_asserts=False, num_devices=n_cores)

    xq_d = nc.dram_tensor("xq", [D, R], BF16, kind="ExternalInput").ap()
    xkT_d = nc.dram_tensor("xkT", [D, NB], BF16, kind="ExternalInput").ap()
    xkl_d = nc.dram_tensor("xkl", [D, LKB * NB], BF16, kind="ExternalInput").ap()
    wq_d = nc.dram_tensor("wq", [D, D], BF16, kind="ExternalInput").ap()
    wk_d = nc.dram_tensor("wk", [D, D], BF16, kind="ExternalInput").ap()
    wvo_d = nc.dram_tensor("wvo", [D, D], BF16, kind="ExternalInput").ap()
    # xn pre-shuffled on host: xn_d[128m+p, 128o+n] = x[128o+p, 128m+n]
    xn_d = nc.dram_tensor("xn", [D, S], BF16, kind="ExternalInput").ap()
    # mask pre-shuffled: mask_d[s, 128t+i] = causal mask for key-tile t,
    # key-in-tile s, chunk-(t//8) query column i (per-core data)
    mask_d = nc.dram_tensor("mask", [P, KT * P], BF16, kind="ExternalInput").ap()
    out_d = nc.dram_tensor("out", [R, D], F32, kind="ExternalOutput").ap()

    def colb(ap_2d, j0, w):
        # DRAM [A, B] column slice [:, j0:j0+w] -> SBUF layout [P, A//P, w]
        return ap_2d[:, j0:j0 + w].rearrange("(o p) n -> p o n", p=P)

    with ExitStack() as ctx:
        tc = ctx.enter_context(tile.TileContext(nc))
        ps_mm = ctx.enter_context(tc.tile_pool(name="ps_mm", bufs=4, space="PSUM"))
        ps_px = ctx.enter_context(tc.tile_pool(name="ps_px", bufs=2, space="PSUM"))
        ps_rs = ctx.enter_context(tc.tile_pool(name="ps_rs", bufs=2, space="PSUM"))
        persist = ctx.enter_context(tc.tile_pool(name="persist", bufs=1))
        dram = ctx.enter_context(tc.tile_pool(name="dram", bufs=1, space="DRAM"))
        stage = ctx.enter_context(tc.tile_pool(name="stage", bufs=2))

        qT = persist.tile([P, DT, R], BF16, tag="qT")
        expb = persist.tile([P, 1], F32, tag="expb")
        nc.vector.memset(expb, EXP_SHIFT)
        ones = persist.tile([P, 1], BF16, tag="ones")
        nc.vector.memset(ones, 1.0)
        rsum = persist.tile([P, NCH], F32, tag="rsum")
        rrec = persist.tile([P, NCH], F32, tag="rrec")

        ks = dram.tile([D, NB], BF16, tag="ks")
        ktg = dram.tile([n_cores * D, NB], BF16, tag="ktg")

        # long-lived mid pool: K-local outputs + mask + first wk reload block
        mid = ctx.enter_context(tc.tile_pool(name="mid", bufs=1))
        kT01 = mid.tile([P, DT, 2 * NB], BF16, tag="kT01")
        blk2 = mid.tile([P, DT, NB], BF16, tag="blk2")
        maskT = mid.tile([P, KT, P], BF16, tag="maskT")
        wkl0 = mid.tile([P, DT, NB], BF16, tag="wkl0")

        # xkl: prefetched under phase 1, consumed by phase 2
        xklp_cm = tc.tile_pool(name="xklp", bufs=1)
        xklp = xklp_cm.__enter__()
        xkl = xklp.tile([P, DT, LKB * NB], BF16, tag="xkl")

        # prefetch chunks paced between late ks shard writes: wk reload
        # block 0 first (K-local starts with it), then the xkl pieces.
        def pf_emit(i):
            if i == 0:
                nc.sync.dma_start(wkl0, colb(wk_d, 0, NB))
            elif i <= LKB:
                h = i - 1
                nc.sync.dma_start(xkl[:, :, h * NB:(h + 1) * NB],
                                  colb(xkl_d, h * NB, NB))

        # ---------------- phase 1: K shard -> DRAM, then AllGather ---------
        # xkT / wk block 0 are loaded in interleaved 4-k-tile chunks so the
        # first accumulation chain starts ~4us in and pipelines behind the
        # (serialized) DMA engines; wk blocks 1-3 are emitted early enough to
        # never stall their chains while keeping the ks writes near the front
        # of the DMA grant order.
        with tc.tile_pool(name="p1", bufs=2) as p1, \
             tc.tile_pool(name="p1x", bufs=1) as p1x:
            xkT = p1x.tile([P, DT, NB], BF16, tag="xkT")
            wkb_a = p1.tile([P, DT, NB], BF16, tag="wkb")
            wkb_b = p1.tile([P, DT, NB], BF16, tag="wkb")
            wkbs = [wkb_a, wkb_b]
            for kc in range(0, DT, 4):
                nc.sync.dma_start(
                    xkT[:, kc:kc + 4, :],
                    xkT_d[kc * P:(kc + 4) * P, :].rearrange("(o p) n -> p o n", p=P))
                nc.sync.dma_start(
                    wkbs[0][:, kc:kc + 4, :],
                    wk_d[kc * P:(kc + 4) * P, 0:NB].rearrange("(o p) n -> p o n", p=P))
            nc.sync.dma_start(wkbs[1], colb(wk_d, NB, NB))
            for mb in range(4):
                if mb >= 2:
                    wkb = p1.tile([P, DT, NB], BF16, tag="wkb")
                    nc.sync.dma_start(wkb, colb(wk_d, mb * NB, NB))
                else:
                    wkb = wkbs[mb]
                for mm in range(4):
                    m = 4 * mb + mm
                    ps = ps_mm.tile([P, NB], F32, tag="mm")
                    for k in range(DT):
                        nc.tensor.matmul(ps, wkb[:, k, mm * P:(mm + 1) * P],
                                         xkT[:, k, :],
                                         start=(k == 0), stop=(k == DT - 1))
                    st = stage.tile([P, NB], BF16, tag="stg")
                    nc.scalar.activation(st, ps, AF.Copy)
                    nc.sync.dma_start(ks[m * P:(m + 1) * P, :], st)
                    if m >= 8 and m % 2 == 0:
                        pf_emit((m - 8) // 2)

        nc.gpsimd.collective_compute(
            "AllGather", mybir.AluOpType.bypass,
            replica_groups=[list(range(n_cores))],
            ins=[ks.opt()], outs=[ktg.opt()],
        )

        # ---------------- phase 2 (under AG): local K blocks 0..2 ----------
        with tc.tile_pool(name="wklp", bufs=2) as wklp:
            for mb in range(4):
                if mb == 0:
                    wkb = wkl0
                else:
                    wkb = wklp.tile([P, DT, NB], BF16, tag="wkl")
                    nc.sync.dma_start(wkb, colb(wk_d, mb * NB, NB))
                for mm in range(4):
                    m = 4 * mb + mm
                    for h in range(LKB):
                        ps = ps_mm.tile([P, NB], F32, tag="mm")
                        for k in range(DT):
                            nc.tensor.matmul(ps, wkb[:, k, mm * P:(mm + 1) * P],
                                             xkl[:, k, h * NB:(h + 1) * NB],
                                             start=(k == 0), stop=(k == DT - 1))
                        dst = kT01[:, m, h * NB:(h + 1) * NB] if h < 2 \
                            else blk2[:, m, :]
                        nc.scalar.activation(dst, ps, AF.Copy)
        xklp_cm.__exit__(None, None, None)

        # ---------------- phase 3 (under AG): Q projection -----------------
        with tc.tile_pool(name="p3", bufs=2) as p3, \
             tc.tile_pool(name="p3x", bufs=1) as p3x:
            xq = p3x.tile([P, DT, R], BF16, tag="xq")
            nc.sync.dma_start(xq, xq_d.rearrange("(o p) n -> p o n", p=P))
            for mb in range(4):
                wqb = p3.tile([P, DT, NB], BF16, tag="wqb")
                nc.sync.dma_start(wqb, colb(wq_d, mb * NB, NB))
                for mm in range(4):
                    m = 4 * mb + mm
                    ps = ps_mm.tile([P, NB], F32, tag="mm")
                    for k in range(DT):
                        nc.tensor.matmul(ps, wqb[:, k, mm * P:(mm + 1) * P],
                                         xq[:, k, :],
                                         start=(k == 0), stop=(k == DT - 1))
                    nc.scalar.activation(qT[:, m, :], ps, AF.Copy)

        # remaining prefetches (issue under AG)
        nc.sync.dma_start(maskT, mask_d.rearrange("p (o n) -> p o n", n=P))
        xnp = ctx.enter_context(tc.tile_pool(name="xnp", bufs=2))
        xn_pre = []
        for m in range(2):
            xb = xnp.tile([P, KT, P], BF16, tag="xn")
            nc.sync.dma_start(
                xb, xn_d[m * P:(m + 1) * P, :].rearrange("p (o n) -> p o n", n=P))
            xn_pre.append(xb)

        late = ctx.enter_context(tc.tile_pool(name="late", bufs=1))
        wT = late.tile([P, N_WBLK * P], BF16, tag="wT")
        pxT = late.tile([P, DT, R], BF16, tag="pxT")

        # staging for gathered kT blocks (b >= 3)
        p4k_cm = tc.tile_pool(name="p4k", bufs=3)
        p4k = p4k_cm.__enter__()

        # ---------------- phase 4: scoresT -> exp -> mask, + row sums ------
        # scoresT[key, q] per key-tile t; chunk j attends tiles t < 8(j+1),
        # so tile t covers query columns [128*(t//8), 512), stored packed at
        # wT column block OFFS[t].
        ktb = None
        for t in range(KT):
            if t >= 4 * LKB and t % 4 == 0:
                b = t // 4
                ktb = p4k.tile([P, DT, NB], BF16, tag="ktb")
                if b == LKB:
                    for kc in range(0, DT, 4):
                        nc.sync.dma_start(
                            ktb[:, kc:kc + 4, :],
                            ktg[b * D + kc * P:b * D + (kc + 4) * P, :]
                            .rearrange("(o p) n -> p o n", p=P))
                else:
                    nc.sync.dma_start(
                        ktb, ktg[b * D:(b + 1) * D, :].rearrange("(o p) n -> p o n", p=P))
            q0 = (t // 8) * P
            w = R - q0
            c0 = OFFS[t] * P
            ps = ps_mm.tile([P, NB], F32, tag="mm")
            for k in range(DT):
                if t < 8:
                    lhs = kT01[:, k, t * P:(t + 1) * P]
                elif t < 4 * LKB:
                    lhs = blk2[:, k, (t % 4) * P:(t % 4 + 1) * P]
                else:
                    lhs = ktb[:, k, (t % 4) * P:(t % 4 + 1) * P]
                nc.tensor.matmul(ps[:, :w], lhs, qT[:, k, q0:R],
                                 start=(k == 0), stop=(k == DT - 1))
            nc.scalar.activation(wT[:, c0:c0 + w], ps[:, :w], AF.Exp, bias=expb)
            nc.vector.tensor_mul(wT[:, c0:c0 + P], wT[:, c0:c0 + P], maskT[:, t, :])
            if t % 8 == 7:
                j = t // 8
                nt = 8 * (j + 1)
                psr = ps_rs.tile([P, 1], F32, tag="rs")
                for tt in range(nt):
                    cb = (OFFS[tt] + j - tt // 8) * P
                    nc.tensor.matmul(psr, wT[:, cb:cb + P], ones,
                                     start=(tt == 0), stop=(tt == nt - 1))
                nc.scalar.activation(rsum[:, j:j + 1], psr, AF.Copy)
        nc.vector.reciprocal(rrec, rsum)
        p4k_cm.__exit__(None, None, None)

        # ---------------- phase 5: pxT = (weights @ x)^T -------------------
        for m in range(DT):
            if m < 2:
                xb = xn_pre[m]
            else:
                xb = xnp.tile([P, KT, P], BF16, tag="xn")
                nc.sync.dma_start(
                    xb, xn_d[m * P:(m + 1) * P, :].rearrange("p (o n) -> p o n", n=P))
            for j in range(NCH):
                nt = 8 * (j + 1)
                ps = ps_px.tile([P, P], F32, tag="px")
                for t in range(nt):
                    cb = (OFFS[t] + j - t // 8) * P
                    nc.tensor.matmul(ps, xb[:, t, :], wT[:, cb:cb + P],
                                     start=(t == 0), stop=(t == nt - 1))
                nc.scalar.activation(pxT[:, m, j * P:(j + 1) * P], ps, AF.Copy)

        # ---------------- phase 6: out = pxT^T @ W_VO, scaled by 1/rowsum --
        with tc.tile_pool(name="wvop", bufs=2) as wvop, \
             tc.tile_pool(name="p7s", bufs=2) as p7s:
            wvo_pre = []
            for nb in range(2):
                wb = wvop.tile([P, DT, NB], BF16, tag="wvo")
                nc.sync.dma_start(wb, colb(wvo_d, nb * NB, NB))
                wvo_pre.append(wb)
            for nb in range(4):
                if nb < 2:
                    wb = wvo_pre[nb]
                else:
                    wb = wvop.tile([P, DT, NB], BF16, tag="wvo")
                    nc.sync.dma_start(wb, colb(wvo_d, nb * NB, NB))
                for j in range(NCH):
                    ps = ps_mm.tile([P, NB], F32, tag="mm")
                    for m in range(DT):
                        nc.tensor.matmul(ps, pxT[:, m, j * P:(j + 1) * P],
                                         wb[:, m, :],
                                         start=(m == 0), stop=(m == DT - 1))
                    ost = p7s.tile([P, NB], F32, tag="ost")
                    nc.scalar.activation(ost, ps, AF.Copy, scale=rrec[:, j:j + 1])
                    nc.sync.dma_start(
                        out_d[j * P:(j + 1) * P, nb * NB:(nb + 1) * NB], ost)

    split_multi_waits(nc)
    return nc


def _rows_of(c):
    return np.concatenate(
        [np.arange(P * (8 * j + c), P * (8 * j + c) + P) for j in range(NCH)])


def _mask_for(c):
    tiles = np.zeros((KT, P, P), dtype=np.float32)
    tri = np.triu(np.ones((P, P), dtype=np.float32))  # [s, i]: 1 iff s <= i
    for t in range(KT):
        u = t % 8
        if u < c:
            tiles[t] = 1.0
        elif u == c:
            tiles[t] = tri
    # -> mask_d[s, 128t + i]
    return tiles.transpose(1, 0, 2).reshape(P, KT * P)


def kernel(x, W_Q, W_K, W_V, W_O, b_Q, b_K, b_V, b_O, d_head, trace=False):
    global LAST_RESULT
    x = np.asarray(x, np.float32)
    n_cores = 8
    dh = float(np.asarray(d_head))
    scale = 1.0 / math.sqrt(dh)
    assert x.shape == (S, D)
    bq = np.asarray(b_Q, np.float32)
    assert not np.any(bq), "b_Q != 0 not supported by this kernel"
    # b_K shifts every score in a row by the same q_i.b_K: softmax-invariant.

    bf = ml_dtypes.bfloat16
    xT_b = np.ascontiguousarray(x.T).astype(bf)                      # [D, S]
    x_b = x.astype(bf)                                               # [S, D]
    wq_b = (np.asarray(W_Q, np.float32) * scale).astype(bf)
    wk_b = np.asarray(W_K, np.float32).astype(bf)
    wvo_b = (np.asarray(W_V, np.float32) @ np.asarray(W_O, np.float32)).astype(bf)
    # xn_d[128m+p, 128o+n] = x[128o+p, 128m+n]
    xn_sh = np.ascontiguousarray(
        x_b.reshape(KT, P, DT, P).transpose(2, 1, 0, 3).reshape(D, S))

    in_maps = []
    for c in range(n_cores):
        in_maps.append({
            "xq": np.ascontiguousarray(xT_b[:, _rows_of(c)]),
            "xkT": np.ascontiguousarray(xT_b[:, c * NB:(c + 1) * NB]),
            "xkl": np.ascontiguousarray(xT_b[:, :LKB * NB]),
            "wq": wq_b, "wk": wk_b, "wvo": wvo_b,
            "xn": xn_sh,
            "mask": _mask_for(c).astype(bf),
        })

    nc = build_bass(n_cores)
    res = bass_utils.run_bass_kernel_spmd(nc, in_maps, core_ids=list(range(n_cores)),
                                          trace=trace)
    LAST_RESULT = res
    out = np.empty((S, D), dtype=np.float32)
    for c in range(n_cores):
        out[_rows_of(c)] = res.results[c]["out"]
    # b_V/b_O fold linearly into the output (softmax rows sum to 1).
    out += (np.asarray(b_V, np.float32) @ np.asarray(W_O, np.float32)
            + np.asarray(b_O, np.float32))[None, :]
    return out
